# revision 15
# baseline (speedup 1.0000x reference)
"""Trainium2 Bass kernel for nn_MetaUpSample (2x meta-upsample, 3x3 dynamic filters).

out[b,ho,wo,f] = sum_k patches[b,ho,wo,k] * meta_w[b,ho,wo,k*3+f]
  patches[b,ho,wo,(dk0,dk1,c)] = x_pad[b, ho//2+dk0, wo//2+dk1, c]

Sharding: 8 cores, core ci handles b = ci//2, ho in [(ci%2)*64, (ci%2)*64+64).

v3 design (baseline f32/DVE-only was 193us; bf16/DVE-only 155us):
  - meta_w and the patch-row tensor xrb stream as BF16 (tolerance 2e-2, bf16
    lands ~2.4e-3), halving the dominant HBM stream 56.6 -> 28.3 MiB/core.
  - host pre-transposes mw to [WO, HO_PC*F*K] so a tile DMA is a plain 2D
    contiguous copy; tile DMAs alternate between the SP and ACT queues.
  - compute pipelines across THREE engines (DVE alone is 132us of STT):
      * DVE: fused scalar_tensor_tensor (mult+accum) for K_VF ops per 12,
        plus batched bf16 tensor_tensor multiplies (2x DVE mode) for K_MV;
      * Pool/GpSimd: batched bf16 tensor_tensor multiplies for the rest
        (walrus rejects TensorScalarPtr on Pool; tensor_tensor compiles);
      * ACT: activation(Copy, accum_out) reduces every non-fused product.
    Products flow through a PRB-deep per-tile ring; reduces of tile t gate
    producers of tile t+PRB (cmp_a), mw slot reuse gates on all three
    reader sems (cmp_v / vm_sem / pm_sem).
"""
from contextlib import ExitStack

import numpy as np

import concourse.bass as bass
import concourse.mybir as mybir
from concourse.bass_utils import run_bass_kernel_spmd

B, H, W, C = 4, 64, 64, 64
HO, WO, F = 128, 128, 3
KS = 3
K = KS * KS * C            # 576
QF = K * F                 # 1728 meta_w channels
RW = KS * C                # 192 floats per patch row (dk1, c)
N_CORES = 8
CORES_PER_B = N_CORES // B         # 2
HO_PC = HO // CORES_PER_B          # 64 output rows per core
NHS = HO_PC // 2                   # 32 hs tiles per core
NROWS = NHS + 2                    # 34 cached padded x rows per core

import os

NBUF = int(os.environ.get("K_NBUF", "4"))    # meta_w double-buffer slots
RPT = int(os.environ.get("K_RPT", "4"))      # meta_w rows per DMA tile
NSCR = int(os.environ.get("K_NSCR", "2"))    # fused-op scratch ring slots
PRB = int(os.environ.get("K_PRB", "4"))      # product ring depth (tiles)
VF12 = int(os.environ.get("K_VF", "6"))      # fused-on-DVE ops per 12
MV12 = int(os.environ.get("K_MV", "6"))      # DVE-mult ops per 12 (rest: Pool)
TAILF = int(os.environ.get("K_TAILF", "2"))  # last tiles fully fused
ALT = os.environ.get("K_ALT", "1") == "1"    # alternate SP/ACT mw queues
# xrb head rows in the first chunk; XH=8 covers compute tiles 0-2 so the
# rest (issued on SP after tile 0) always lands before it is needed
XH = int(os.environ.get("K_XH", "8"))

f32 = mybir.dt.float32
bf16 = mybir.dt.bfloat16

if os.environ.get("K_RAMP", "1") == "1":
    # small head tiles: the first DVE op only needs a 0.44MiB transfer, so
    # compute starts ~8us earlier; small tail tiles shorten the drain.
    SCHED = [1, 1, 2] + [RPT] * ((HO_PC - 8) // RPT) + [2, 1, 1]
elif os.environ.get("K_TAIL", "1") == "1":
    SCHED = [RPT] * ((HO_PC - 4) // RPT) + [2, 1, 1]
else:
    SCHED = [RPT] * (HO_PC // RPT)
assert sum(SCHED) == HO_PC
NT = len(SCHED)
ROW0 = [sum(SCHED[:t]) for t in range(NT)]          # first ho row of tile t
MAXOPS = max(SCHED) * F

# ---------------------------------------------------------------------------
# Static per-tile plan: op o = (r, f) at slot offset o*K; first VF ops fused
# on DVE, next MV multiplied on DVE, rest multiplied on Pool; every non-fused
# op is reduced on ACT. Mult batches group consecutive ops sharing a window.
PLAN = []      # per tile: dict(vf, mv_batches, mp_batches, n)
for t in range(NT):
    n = SCHED[t] * F
    if t >= NT - TAILF:
        vf, mv = n, 0
    else:
        vf = min(n, (n * VF12 + 6) // 12)
        mv = min(n - vf, (n * MV12 + 6) // 12)

    def _batches(lo, hi):
        """group ops [lo, hi) into runs sharing a window"""
        runs = []
        o = lo
        while o < hi:
            m = (ROW0[t] + o // F) // 2
            e = o
            while e < hi and (ROW0[t] + e // F) // 2 == m:
                e += 1
            runs.append((o, e, m))
            o = e
        return runs

    PLAN.append(
        dict(
            n=n,
            vf=vf,
            mv=mv,
            mvb=_batches(vf, vf + mv),
            mpb=_batches(vf + mv, n),
        )
    )

VF_CUM = [0]   # fused DVE ops completed through tile t-1
VMB_CUM = [0]  # DVE mult batches
PMB_CUM = [0]  # Pool mult batches
A_CUM = [0]    # ACT reduces
for t in range(NT):
    p = PLAN[t]
    VF_CUM.append(VF_CUM[-1] + p["vf"])
    VMB_CUM.append(VMB_CUM[-1] + len(p["mvb"]))
    PMB_CUM.append(PMB_CUM[-1] + len(p["mpb"]))
    A_CUM.append(A_CUM[-1] + (p["n"] - p["vf"]))

# tiles covering rows [0, 60): used for the early output-store overlap
T60 = next(t for t in range(NT + 1) if sum(SCHED[:t]) >= 60)

_CACHED = None


def _build_nc():
    # Cross-engine ordering is fully explicit via semaphores below; the
    # remaining same-engine WAW (scratch rings) is safe on HW because each
    # engine drains its pipe between ops, so skip the detector's extra waits.
    nc = bass.Bass(detect_race_conditions=False)
    mw_d = nc.declare_dram_parameter("mw", [WO, HO_PC * QF], bf16, isOutput=False)
    xrb_d = nc.declare_dram_parameter("xrb", [WO, NROWS * RW], bf16, isOutput=False)
    out_d = nc.declare_dram_parameter("out", [WO, HO_PC * F], f32, isOutput=True)

    with ExitStack() as ctx:
        xrow = ctx.enter_context(nc.sbuf_tensor([WO, NROWS * RW], bf16))
        mwbuf = ctx.enter_context(nc.sbuf_tensor([WO, NBUF * RPT * QF], bf16))
        prod = ctx.enter_context(nc.sbuf_tensor([WO, PRB * MAXOPS * K], bf16))
        scr_v = ctx.enter_context(nc.sbuf_tensor([WO, NSCR * K], bf16))
        ascr = ctx.enter_context(nc.sbuf_tensor([WO, NSCR * K], bf16))
        out_sb = ctx.enter_context(nc.sbuf_tensor([WO, HO_PC * F], f32))
        slot_sem = [ctx.enter_context(nc.semaphore(f"slot{j}")) for j in range(NBUF)]
        xh_sem = ctx.enter_context(nc.semaphore("xh"))      # xrb head landed
        misc_sem = ctx.enter_context(nc.semaphore("misc"))  # xrb rest landed
        cmp_v = ctx.enter_context(nc.semaphore("cmp_v"))   # DVE fused ops done
        cmp_a = ctx.enter_context(nc.semaphore("cmp_a"))   # ACT reduces done
        vm_sem = ctx.enter_context(nc.semaphore("vm"))     # DVE mult batches
        pm_sem = ctx.enter_context(nc.semaphore("pm"))     # Pool mult batches
        # Pool's dge_drain at block exit is expensive; skip it when no pool
        # ops are emitted (pm_sem waits order everything that matters).
        block = ctx.enter_context(nc.Block(no_gpsimd_drain=PMB_CUM[NT] == 0))

        def slot_ap(j, rows):
            base = j * RPT * QF
            return mwbuf[:, base : base + rows * QF]

        def win_ap(m, nb):
            w = xrow[:, m * RW : m * RW + KS * RW]
            if nb == 1:
                return w.rearrange("p (o k) -> p o k", o=1)
            return w.rearrange("p (o k) -> p o k", o=1).to_broadcast([WO, nb, K])

        def prod_op_ap(t, o):
            base = (t % PRB) * MAXOPS * K + o * K
            return prod[:, base : base + K]

        def prod_run_ap(t, o0, o1):
            base = (t % PRB) * MAXOPS * K
            return prod[:, base + o0 * K : base + o1 * K].rearrange(
                "p (o k) -> p o k", o=o1 - o0
            )

        def issue_tile(eng, i):
            j = i % NBUF
            rows, row0 = SCHED[i], ROW0[i]
            if i >= NBUF:
                # all three mw readers finished with the slot's previous tile
                prev = i - NBUF
                if VF_CUM[prev + 1] > 0:
                    eng.wait_ge(cmp_v, VF_CUM[prev + 1])
                if VMB_CUM[prev + 1] > 0:
                    eng.wait_ge(vm_sem, VMB_CUM[prev + 1])
                if PMB_CUM[prev + 1] > 0:
                    eng.wait_ge(pm_sem, PMB_CUM[prev + 1])
            eng.dma_start(
                out=slot_ap(j, rows),
                in_=mw_d[:, row0 * QF : (row0 + rows) * QF],
            ).then_inc(slot_sem[j], 16)

        def act_owned(i):
            return ALT and i % 2 == 1

        @block.sync
        def _(sync):
            first = True
            for i in range(NT):
                if act_owned(i):
                    continue
                issue_tile(sync, i)
                if first:
                    # xrb rest right behind tile 0: lands ~10us in, well
                    # before compute reaches rows beyond the XH-row head
                    sync.dma_start(
                        out=xrow[:, XH * RW :], in_=xrb_d[:, XH * RW :]
                    ).then_inc(misc_sem, 16)
                    first = False
            # overlap the bulk of the output store with the tail tiles
            if VF_CUM[T60] > 0:
                sync.wait_ge(cmp_v, VF_CUM[T60])
            if A_CUM[T60] > 0:
                sync.wait_ge(cmp_a, A_CUM[T60])
            sync.dma_start(
                out=out_d[:, : 60 * F], in_=out_sb[:, : 60 * F]
            ).then_inc(misc_sem, 16)
            if VF_CUM[NT] > 0:
                sync.wait_ge(cmp_v, VF_CUM[NT])
            if A_CUM[NT] > 0:
                sync.wait_ge(cmp_a, A_CUM[NT])
            sync.dma_start(
                out=out_d[:, 60 * F :], in_=out_sb[:, 60 * F :]
            ).then_inc(misc_sem, 16)

        @block.scalar
        def _(scalar):
            # xrb head first so compute can start after ~0.4MiB; ACT-owned
            # tiles < NBUF next (no wait conditions).
            scalar.dma_start(
                out=xrow[:, : XH * RW], in_=xrb_d[:, : XH * RW]
            ).then_inc(xh_sem, 16)
            for i in range(min(NBUF, NT)):
                if act_owned(i):
                    issue_tile(scalar, i)
            # interleave reduces with this queue's remaining tile issues
            vm_seen = pm_seen = 0
            na = 0
            for t in range(NT):
                p = PLAN[t]
                # pool products land earliest; reduce them first, then DVE's
                order = [(o0, o1, "p") for (o0, o1, _m) in p["mpb"]] + [
                    (o0, o1, "v") for (o0, o1, _m) in p["mvb"]
                ]
                for o0, o1, src in order:
                    if src == "p":
                        pm_seen += 1
                        scalar.wait_ge(pm_sem, pm_seen)
                    else:
                        vm_seen += 1
                        scalar.wait_ge(vm_sem, vm_seen)
                    for o in range(o0, o1):
                        r, f = divmod(o, F)
                        ho = ROW0[t] + r
                        scalar.activation(
                            out=ascr[:, (na % NSCR) * K : (na % NSCR + 1) * K],
                            in_=prod_op_ap(t, o),
                            func=mybir.ActivationFunctionType.Copy,
                            accum_out=out_sb[:, ho * F + f : ho * F + f + 1],
                        ).then_inc(cmp_a, 1)
                        na += 1
                i2 = t + NBUF
                if i2 < NT and act_owned(i2):
                    issue_tile(scalar, i2)

        @block.vector
        def _(vector):
            vector.wait_ge(xh_sem, 16)
            nv = 0
            xrow_full_waited = False
            for t in range(NT):
                p = PLAN[t]
                j, pg = t % NBUF, t // NBUF
                rows = SCHED[t]
                if (
                    not xrow_full_waited
                    and (ROW0[t] + rows - 1) // 2 + 2 >= XH
                ):
                    vector.wait_ge(misc_sem, 16)  # rest of xrow loaded
                    xrow_full_waited = True
                vector.wait_ge(slot_sem[j], 16 * (pg + 1))
                mw3 = slot_ap(j, rows).rearrange("p (o k) -> p o k", o=rows * F)
                # multiplies first: ACT can start this tile's reduces while
                # the fused ops below are still running
                if p["mvb"]:
                    if t >= PRB and A_CUM[t - PRB + 1] > 0:
                        vector.wait_ge(cmp_a, A_CUM[t - PRB + 1])
                    for o0, o1, m in p["mvb"]:
                        vector.tensor_tensor(
                            out=prod_run_ap(t, o0, o1),
                            in0=mw3[:, o0:o1, :],
                            in1=win_ap(m, o1 - o0),
                            op=mybir.AluOpType.mult,
                        ).then_inc(vm_sem, 1)
                for o in range(p["vf"]):
                    r, f = divmod(o, F)
                    ho = ROW0[t] + r
                    vector.scalar_tensor_tensor(
                        out=scr_v[:, (nv % NSCR) * K : (nv % NSCR + 1) * K],
                        in0=mw3[:, o, :],
                        scalar=1.0,
                        in1=xrow[:, (ho // 2) * RW : (ho // 2) * RW + KS * RW],
                        op0=mybir.AluOpType.mult,
                        op1=mybir.AluOpType.mult,
                        accum_out=out_sb[:, ho * F + f : ho * F + f + 1],
                    ).then_inc(cmp_v, 1)
                    nv += 1

        if PMB_CUM[NT] > 0:

            @block.gpsimd
            def _(gpsimd):
                gpsimd.wait_ge(xh_sem, 16)
                xrow_full_waited = False
                for t in range(NT):
                    p = PLAN[t]
                    if not p["mpb"]:
                        continue
                    j, pg = t % NBUF, t // NBUF
                    rows = SCHED[t]
                    if (
                        not xrow_full_waited
                        and (ROW0[t] + rows - 1) // 2 + 2 >= XH
                    ):
                        gpsimd.wait_ge(misc_sem, 16)
                        xrow_full_waited = True
                    gpsimd.wait_ge(slot_sem[j], 16 * (pg + 1))
                    if t >= PRB and A_CUM[t - PRB + 1] > 0:
                        gpsimd.wait_ge(cmp_a, A_CUM[t - PRB + 1])
                    mw3 = slot_ap(j, rows).rearrange(
                        "p (o k) -> p o k", o=rows * F
                    )
                    for o0, o1, m in p["mpb"]:
                        gpsimd.tensor_tensor(
                            out=prod_run_ap(t, o0, o1),
                            in0=mw3[:, o0:o1, :],
                            in1=win_ap(m, o1 - o0),
                            op=mybir.AluOpType.mult,
                        ).then_inc(pm_sem, 1)

    return nc


def _prep_xrb(x):
    """Per-core duplicated patch-row tensors (bf16).

    xrb[ci][wo, hpl*RW + dk1*C + c] = x_pad[b, hs0+hpl, wo//2 + dk1, c]
    where x_pad has 1 zero row/col of padding on each side.
    """
    import ml_dtypes
    from numpy.lib.stride_tricks import sliding_window_view

    out = []
    for ci in range(N_CORES):
        b, hs0 = ci // CORES_PER_B, (ci % CORES_PER_B) * NHS
        xp = np.pad(x[b], ((1, 1), (1, 1), (0, 0)))          # [66, 66, 64]
        rows = xp[hs0 : hs0 + NROWS]                          # [34, 66, 64]
        win = sliding_window_view(rows, KS, axis=1)           # [34, 64(ws), 64(c), 3(dk1)]
        win = win.transpose(0, 1, 3, 2).reshape(NROWS, W, RW)  # [34, 64, 192]
        dup = np.repeat(win, 2, axis=1)                       # [34, 128, 192]
        out.append(
            np.ascontiguousarray(dup.transpose(1, 0, 2))
            .reshape(WO, NROWS * RW)
            .astype(ml_dtypes.bfloat16)
        )
    return out


def _ensure_axon_hooks_module():
    """This image's antenv lacks axon_hooks; run_bass_kernel_spmd imports it
    when BASS_TRACE is set. Provide it (registering the real NTFF hook when
    available) so tracing degrades gracefully instead of crashing."""
    try:
        import antenv.axon_hooks  # noqa: F401
        return
    except ImportError:
        pass
    import sys
    import types

    try:
        import antenv
    except ImportError:
        return
    mod = types.ModuleType("antenv.axon_hooks")
    _hook = [None]
    mod.set_axon_ntff_profile_hook = lambda h: _hook.__setitem__(0, h)
    mod.get_axon_ntff_profile_hook = lambda: _hook[0]
    sys.modules["antenv.axon_hooks"] = mod
    antenv.axon_hooks = mod
    try:
        from trn_agent_boot.trn_boot import _ntff_profile_via_ctypes

        h = _ntff_profile_via_ctypes("/opt/axon/libaxon_pjrt.so")
        if h is not None:
            _hook[0] = h
    except Exception:
        pass


_ensure_axon_hooks_module()

last_results = None  # BassKernelResults of the most recent kernel() call


def kernel(x, meta_w):
    global _CACHED, last_results
    import ml_dtypes

    x = np.ascontiguousarray(np.asarray(x, dtype=np.float32))
    meta_w = np.asarray(meta_w, dtype=np.float32)

    if _CACHED is None:
        _CACHED = _build_nc()
    nc = _CACHED

    xrbs = _prep_xrb(x)
    mw16 = meta_w.astype(ml_dtypes.bfloat16)
    in_maps = []
    for ci in range(N_CORES):
        b, ho0 = ci // CORES_PER_B, (ci % CORES_PER_B) * HO_PC
        # [HO_PC, WO, K, F] -> [WO, HO_PC, F, K] so a tile DMA is one plain
        # 2D contiguous copy and the per-op in0 read is contiguous (F-major).
        mw_c = np.ascontiguousarray(
            mw16[b, ho0 : ho0 + HO_PC].reshape(HO_PC, WO, K, F).transpose(1, 0, 3, 2)
        ).reshape(WO, HO_PC * QF)
        in_maps.append({"mw": mw_c, "xrb": xrbs[ci]})

    res = run_bass_kernel_spmd(nc, in_maps, list(range(N_CORES)))
    last_results = res

    out = np.empty((B, HO, WO, F), np.float32)
    for ci in range(N_CORES):
        b, ho0 = ci // CORES_PER_B, (ci % CORES_PER_B) * HO_PC
        o = res.results[ci]["out"].reshape(WO, HO_PC, F)
        out[b, ho0 : ho0 + HO_PC] = o.transpose(1, 0, 2)
    return out


# revision 17
# speedup vs baseline: 1.1670x; 1.1670x over previous
"""Trainium2 Bass kernel for nn_MetaUpSample (2x meta-upsample, 3x3 dynamic filters).

out[b,ho,wo,f] = sum_k patches[b,ho,wo,k] * meta_w[b,ho,wo,k*3+f]
  patches[b,ho,wo,(dk0,dk1,c)] = x_pad[b, ho//2+dk0, wo//2+dk1, c]

Sharding: 8 cores, core ci handles b = ci//2, ho in [(ci%2)*64, (ci%2)*64+64).

v3 design (baseline f32/DVE-only was 193us; bf16/DVE-only 155us):
  - meta_w and the patch-row tensor xrb stream as BF16 (tolerance 2e-2, bf16
    lands ~2.4e-3), halving the dominant HBM stream 56.6 -> 28.3 MiB/core.
  - host pre-transposes mw to [WO, HO_PC*F*K] so a tile DMA is a plain 2D
    contiguous copy; tile DMAs alternate between the SP and ACT queues.
  - compute pipelines across THREE engines (DVE alone is 132us of STT):
      * DVE: fused scalar_tensor_tensor (mult+accum) for K_VF ops per 12,
        plus batched bf16 tensor_tensor multiplies (2x DVE mode) for K_MV;
      * Pool/GpSimd: batched bf16 tensor_tensor multiplies for the rest
        (walrus rejects TensorScalarPtr on Pool; tensor_tensor compiles);
      * ACT: activation(Copy, accum_out) reduces every non-fused product.
    Products flow through a PRB-deep per-tile ring; reduces of tile t gate
    producers of tile t+PRB (cmp_a), mw slot reuse gates on all three
    reader sems (cmp_v / vm_sem / pm_sem).
"""
from contextlib import ExitStack

import numpy as np

import concourse.bass as bass
import concourse.mybir as mybir
from concourse.bass_utils import run_bass_kernel_spmd

B, H, W, C = 4, 64, 64, 64
HO, WO, F = 128, 128, 3
KS = 3
K = KS * KS * C            # 576
QF = K * F                 # 1728 meta_w channels
RW = KS * C                # 192 floats per patch row (dk1, c)
N_CORES = 8
CORES_PER_B = N_CORES // B         # 2
HO_PC = HO // CORES_PER_B          # 64 output rows per core
NHS = HO_PC // 2                   # 32 hs tiles per core
NROWS = NHS + 2                    # 34 cached padded x rows per core

import os

NBUF = int(os.environ.get("K_NBUF", "4"))    # meta_w double-buffer slots
RPT = int(os.environ.get("K_RPT", "4"))      # meta_w rows per DMA tile
NSCR = int(os.environ.get("K_NSCR", "2"))    # fused-op scratch ring slots
PRB = int(os.environ.get("K_PRB", "4"))      # product ring depth (tiles)
VF12 = int(os.environ.get("K_VF", "6"))      # fused-on-DVE ops per 12
MV12 = int(os.environ.get("K_MV", "6"))      # DVE-mult ops per 12 (rest: Pool)
TAILF = int(os.environ.get("K_TAILF", "2"))  # last tiles fully fused
ALT = os.environ.get("K_ALT", "1") == "1"    # alternate SP/ACT mw queues
MFIRST = os.environ.get("K_MFIRST", "1") == "1"  # DVE mults before fused ops
# xrb head rows in the first chunk; XH=8 covers compute tiles 0-2 so the
# rest (issued on SP after tile 0) always lands before it is needed
XH = int(os.environ.get("K_XH", "8"))

f32 = mybir.dt.float32
bf16 = mybir.dt.bfloat16

if os.environ.get("K_RAMP", "1") == "1":
    # small head tiles: the first DVE op only needs a 0.44MiB transfer, so
    # compute starts ~8us earlier; small tail tiles shorten the drain.
    SCHED = [1, 1, 2] + [RPT] * ((HO_PC - 8) // RPT) + [2, 1, 1]
elif os.environ.get("K_TAIL", "1") == "1":
    SCHED = [RPT] * ((HO_PC - 4) // RPT) + [2, 1, 1]
else:
    SCHED = [RPT] * (HO_PC // RPT)
assert sum(SCHED) == HO_PC
NT = len(SCHED)
ROW0 = [sum(SCHED[:t]) for t in range(NT)]          # first ho row of tile t
MAXOPS = max(SCHED) * F

# ---------------------------------------------------------------------------
# Static per-tile plan: op o = (r, f) at slot offset o*K; first VF ops fused
# on DVE, next MV multiplied on DVE, rest multiplied on Pool; every non-fused
# op is reduced on ACT. Mult batches group consecutive ops sharing a window.
PLAN = []      # per tile: dict(vf, mv_batches, mp_batches, n)
for t in range(NT):
    n = SCHED[t] * F
    if t >= NT - TAILF:
        vf, mv = n, 0
    else:
        vf = min(n, (n * VF12 + 6) // 12)
        mv = min(n - vf, (n * MV12 + 6) // 12)

    def _batches(lo, hi):
        """group ops [lo, hi) into runs sharing a window"""
        runs = []
        o = lo
        while o < hi:
            m = (ROW0[t] + o // F) // 2
            e = o
            while e < hi and (ROW0[t] + e // F) // 2 == m:
                e += 1
            runs.append((o, e, m))
            o = e
        return runs

    PLAN.append(
        dict(
            n=n,
            vf=vf,
            mv=mv,
            mvb=_batches(vf, vf + mv),
            mpb=_batches(vf + mv, n),
        )
    )

VF_CUM = [0]   # fused DVE ops completed through tile t-1
VMB_CUM = [0]  # DVE mult batches
PMB_CUM = [0]  # Pool mult batches
A_CUM = [0]    # ACT reduces
for t in range(NT):
    p = PLAN[t]
    VF_CUM.append(VF_CUM[-1] + p["vf"])
    VMB_CUM.append(VMB_CUM[-1] + len(p["mvb"]))
    PMB_CUM.append(PMB_CUM[-1] + len(p["mpb"]))
    A_CUM.append(A_CUM[-1] + (p["n"] - p["vf"]))

# tiles covering rows [0, 60): used for the early output-store overlap
T60 = next(t for t in range(NT + 1) if sum(SCHED[:t]) >= 60)

_CACHED = None


def _build_nc():
    # Cross-engine ordering is fully explicit via semaphores below; the
    # remaining same-engine WAW (scratch rings) is safe on HW because each
    # engine drains its pipe between ops, so skip the detector's extra waits.
    nc = bass.Bass(detect_race_conditions=False)
    mw_d = nc.declare_dram_parameter("mw", [WO, HO_PC * QF], bf16, isOutput=False)
    xrb_d = nc.declare_dram_parameter("xrb", [WO, NROWS * RW], bf16, isOutput=False)
    out_d = nc.declare_dram_parameter("out", [WO, HO_PC * F], f32, isOutput=True)

    with ExitStack() as ctx:
        xrow = ctx.enter_context(nc.sbuf_tensor([WO, NROWS * RW], bf16))
        mwbuf = ctx.enter_context(nc.sbuf_tensor([WO, NBUF * RPT * QF], bf16))
        prod = ctx.enter_context(nc.sbuf_tensor([WO, PRB * MAXOPS * K], bf16))
        scr_v = ctx.enter_context(nc.sbuf_tensor([WO, NSCR * K], bf16))
        ascr = ctx.enter_context(nc.sbuf_tensor([WO, NSCR * K], bf16))
        out_sb = ctx.enter_context(nc.sbuf_tensor([WO, HO_PC * F], f32))
        slot_sem = [ctx.enter_context(nc.semaphore(f"slot{j}")) for j in range(NBUF)]
        xh_sem = ctx.enter_context(nc.semaphore("xh"))      # xrb head landed
        misc_sem = ctx.enter_context(nc.semaphore("misc"))  # xrb rest landed
        cmp_v = ctx.enter_context(nc.semaphore("cmp_v"))   # DVE fused ops done
        cmp_a = ctx.enter_context(nc.semaphore("cmp_a"))   # ACT reduces done
        vm_sem = ctx.enter_context(nc.semaphore("vm"))     # DVE mult batches
        pm_sem = ctx.enter_context(nc.semaphore("pm"))     # Pool mult batches
        # Pool's dge_drain at block exit is expensive; skip it when no pool
        # ops are emitted (pm_sem waits order everything that matters).
        block = ctx.enter_context(nc.Block(no_gpsimd_drain=PMB_CUM[NT] == 0))

        def slot_ap(j, rows):
            base = j * RPT * QF
            return mwbuf[:, base : base + rows * QF]

        def win_ap(m, nb):
            w = xrow[:, m * RW : m * RW + KS * RW]
            if nb == 1:
                return w.rearrange("p (o k) -> p o k", o=1)
            return w.rearrange("p (o k) -> p o k", o=1).to_broadcast([WO, nb, K])

        def prod_op_ap(t, o):
            base = (t % PRB) * MAXOPS * K + o * K
            return prod[:, base : base + K]

        def prod_run_ap(t, o0, o1):
            base = (t % PRB) * MAXOPS * K
            return prod[:, base + o0 * K : base + o1 * K].rearrange(
                "p (o k) -> p o k", o=o1 - o0
            )

        def issue_tile(eng, i):
            j = i % NBUF
            rows, row0 = SCHED[i], ROW0[i]
            if i >= NBUF:
                # all three mw readers finished with the slot's previous tile
                prev = i - NBUF
                if VF_CUM[prev + 1] > 0:
                    eng.wait_ge(cmp_v, VF_CUM[prev + 1])
                if VMB_CUM[prev + 1] > 0:
                    eng.wait_ge(vm_sem, VMB_CUM[prev + 1])
                if PMB_CUM[prev + 1] > 0:
                    eng.wait_ge(pm_sem, PMB_CUM[prev + 1])
            eng.dma_start(
                out=slot_ap(j, rows),
                in_=mw_d[:, row0 * QF : (row0 + rows) * QF],
            ).then_inc(slot_sem[j], 16)

        def act_owned(i):
            return ALT and i % 2 == 1

        @block.sync
        def _(sync):
            first = True
            for i in range(NT):
                if act_owned(i):
                    continue
                issue_tile(sync, i)
                if first:
                    # xrb rest right behind tile 0: lands ~10us in, well
                    # before compute reaches rows beyond the XH-row head
                    sync.dma_start(
                        out=xrow[:, XH * RW :], in_=xrb_d[:, XH * RW :]
                    ).then_inc(misc_sem, 16)
                    first = False
            # overlap the bulk of the output store with the tail tiles
            if VF_CUM[T60] > 0:
                sync.wait_ge(cmp_v, VF_CUM[T60])
            if A_CUM[T60] > 0:
                sync.wait_ge(cmp_a, A_CUM[T60])
            sync.dma_start(
                out=out_d[:, : 60 * F], in_=out_sb[:, : 60 * F]
            ).then_inc(misc_sem, 16)
            if VF_CUM[NT] > 0:
                sync.wait_ge(cmp_v, VF_CUM[NT])
            if A_CUM[NT] > 0:
                sync.wait_ge(cmp_a, A_CUM[NT])
            sync.dma_start(
                out=out_d[:, 60 * F :], in_=out_sb[:, 60 * F :]
            ).then_inc(misc_sem, 16)

        @block.scalar
        def _(scalar):
            # xrb head first so compute can start after ~0.4MiB; ACT-owned
            # tiles < NBUF next (no wait conditions).
            scalar.dma_start(
                out=xrow[:, : XH * RW], in_=xrb_d[:, : XH * RW]
            ).then_inc(xh_sem, 16)
            for i in range(min(NBUF, NT)):
                if act_owned(i):
                    issue_tile(scalar, i)
            # interleave reduces with this queue's remaining tile issues
            vm_seen = pm_seen = 0
            na = 0
            for t in range(NT):
                p = PLAN[t]
                # pool products land earliest; reduce them first, then DVE's
                order = [(o0, o1, "p") for (o0, o1, _m) in p["mpb"]] + [
                    (o0, o1, "v") for (o0, o1, _m) in p["mvb"]
                ]
                for o0, o1, src in order:
                    if src == "p":
                        pm_seen += 1
                        scalar.wait_ge(pm_sem, pm_seen)
                    else:
                        vm_seen += 1
                        scalar.wait_ge(vm_sem, vm_seen)
                    for o in range(o0, o1):
                        r, f = divmod(o, F)
                        ho = ROW0[t] + r
                        scalar.activation(
                            out=ascr[:, (na % NSCR) * K : (na % NSCR + 1) * K],
                            in_=prod_op_ap(t, o),
                            func=mybir.ActivationFunctionType.Copy,
                            accum_out=out_sb[:, ho * F + f : ho * F + f + 1],
                        ).then_inc(cmp_a, 1)
                        na += 1
                i2 = t + NBUF
                if i2 < NT and act_owned(i2):
                    issue_tile(scalar, i2)

        @block.vector
        def _(vector):
            vector.wait_ge(xh_sem, 16)
            nv = 0
            xrow_full_waited = False
            for t in range(NT):
                p = PLAN[t]
                j, pg = t % NBUF, t // NBUF
                rows = SCHED[t]
                if (
                    not xrow_full_waited
                    and (ROW0[t] + rows - 1) // 2 + 2 >= XH
                ):
                    vector.wait_ge(misc_sem, 16)  # rest of xrow loaded
                    xrow_full_waited = True
                vector.wait_ge(slot_sem[j], 16 * (pg + 1))
                mw3 = slot_ap(j, rows).rearrange("p (o k) -> p o k", o=rows * F)
                def do_mults():
                    if p["mvb"]:
                        if t >= PRB and A_CUM[t - PRB + 1] > 0:
                            vector.wait_ge(cmp_a, A_CUM[t - PRB + 1])
                        for o0, o1, m in p["mvb"]:
                            vector.tensor_tensor(
                                out=prod_run_ap(t, o0, o1),
                                in0=mw3[:, o0:o1, :],
                                in1=win_ap(m, o1 - o0),
                                op=mybir.AluOpType.mult,
                            ).then_inc(vm_sem, 1)

                # multiplies first: ACT can start this tile's reduces while
                # the fused ops below are still running
                if MFIRST:
                    do_mults()
                for o in range(p["vf"]):
                    r, f = divmod(o, F)
                    ho = ROW0[t] + r
                    vector.scalar_tensor_tensor(
                        out=scr_v[:, (nv % NSCR) * K : (nv % NSCR + 1) * K],
                        in0=mw3[:, o, :],
                        scalar=1.0,
                        in1=xrow[:, (ho // 2) * RW : (ho // 2) * RW + KS * RW],
                        op0=mybir.AluOpType.mult,
                        op1=mybir.AluOpType.mult,
                        accum_out=out_sb[:, ho * F + f : ho * F + f + 1],
                    ).then_inc(cmp_v, 1)
                    nv += 1
                if not MFIRST:
                    do_mults()

        if PMB_CUM[NT] > 0:

            @block.gpsimd
            def _(gpsimd):
                gpsimd.wait_ge(xh_sem, 16)
                xrow_full_waited = False
                for t in range(NT):
                    p = PLAN[t]
                    if not p["mpb"]:
                        continue
                    j, pg = t % NBUF, t // NBUF
                    rows = SCHED[t]
                    if (
                        not xrow_full_waited
                        and (ROW0[t] + rows - 1) // 2 + 2 >= XH
                    ):
                        gpsimd.wait_ge(misc_sem, 16)
                        xrow_full_waited = True
                    gpsimd.wait_ge(slot_sem[j], 16 * (pg + 1))
                    if t >= PRB and A_CUM[t - PRB + 1] > 0:
                        gpsimd.wait_ge(cmp_a, A_CUM[t - PRB + 1])
                    mw3 = slot_ap(j, rows).rearrange(
                        "p (o k) -> p o k", o=rows * F
                    )
                    for o0, o1, m in p["mpb"]:
                        gpsimd.tensor_tensor(
                            out=prod_run_ap(t, o0, o1),
                            in0=mw3[:, o0:o1, :],
                            in1=win_ap(m, o1 - o0),
                            op=mybir.AluOpType.mult,
                        ).then_inc(pm_sem, 1)

    return nc


def _prep_xrb(x):
    """Per-core duplicated patch-row tensors (bf16).

    xrb[ci][wo, hpl*RW + dk1*C + c] = x_pad[b, hs0+hpl, wo//2 + dk1, c]
    where x_pad has 1 zero row/col of padding on each side.
    """
    import ml_dtypes
    from numpy.lib.stride_tricks import sliding_window_view

    out = []
    for ci in range(N_CORES):
        b, hs0 = ci // CORES_PER_B, (ci % CORES_PER_B) * NHS
        xp = np.pad(x[b], ((1, 1), (1, 1), (0, 0)))          # [66, 66, 64]
        rows = xp[hs0 : hs0 + NROWS]                          # [34, 66, 64]
        win = sliding_window_view(rows, KS, axis=1)           # [34, 64(ws), 64(c), 3(dk1)]
        win = win.transpose(0, 1, 3, 2).reshape(NROWS, W, RW)  # [34, 64, 192]
        dup = np.repeat(win, 2, axis=1)                       # [34, 128, 192]
        out.append(
            np.ascontiguousarray(dup.transpose(1, 0, 2))
            .reshape(WO, NROWS * RW)
            .astype(ml_dtypes.bfloat16)
        )
    return out


def _ensure_axon_hooks_module():
    """This image's antenv lacks axon_hooks; run_bass_kernel_spmd imports it
    when BASS_TRACE is set. Provide it (registering the real NTFF hook when
    available) so tracing degrades gracefully instead of crashing."""
    try:
        import antenv.axon_hooks  # noqa: F401
        return
    except ImportError:
        pass
    import sys
    import types

    try:
        import antenv
    except ImportError:
        return
    mod = types.ModuleType("antenv.axon_hooks")
    _hook = [None]
    mod.set_axon_ntff_profile_hook = lambda h: _hook.__setitem__(0, h)
    mod.get_axon_ntff_profile_hook = lambda: _hook[0]
    sys.modules["antenv.axon_hooks"] = mod
    antenv.axon_hooks = mod
    try:
        from trn_agent_boot.trn_boot import _ntff_profile_via_ctypes

        h = _ntff_profile_via_ctypes("/opt/axon/libaxon_pjrt.so")
        if h is not None:
            _hook[0] = h
    except Exception:
        pass


_ensure_axon_hooks_module()

last_results = None  # BassKernelResults of the most recent kernel() call


def kernel(x, meta_w):
    global _CACHED, last_results
    import ml_dtypes

    x = np.ascontiguousarray(np.asarray(x, dtype=np.float32))
    meta_w = np.asarray(meta_w, dtype=np.float32)

    if _CACHED is None:
        _CACHED = _build_nc()
    nc = _CACHED

    xrbs = _prep_xrb(x)
    mw16 = meta_w.astype(ml_dtypes.bfloat16)
    in_maps = []
    for ci in range(N_CORES):
        b, ho0 = ci // CORES_PER_B, (ci % CORES_PER_B) * HO_PC
        # [HO_PC, WO, K, F] -> [WO, HO_PC, F, K] so a tile DMA is one plain
        # 2D contiguous copy and the per-op in0 read is contiguous (F-major).
        mw_c = np.ascontiguousarray(
            mw16[b, ho0 : ho0 + HO_PC].reshape(HO_PC, WO, K, F).transpose(1, 0, 3, 2)
        ).reshape(WO, HO_PC * QF)
        in_maps.append({"mw": mw_c, "xrb": xrbs[ci]})

    res = run_bass_kernel_spmd(nc, in_maps, list(range(N_CORES)))
    last_results = res

    out = np.empty((B, HO, WO, F), np.float32)
    for ci in range(N_CORES):
        b, ho0 = ci // CORES_PER_B, (ci % CORES_PER_B) * HO_PC
        o = res.results[ci]["out"].reshape(WO, HO_PC, F)
        out[b, ho0 : ho0 + HO_PC] = o.transpose(1, 0, 2)
    return out


# revision 20
# speedup vs baseline: 1.2014x; 1.0295x over previous
"""Trainium2 Bass kernel for nn_MetaUpSample (2x meta-upsample, 3x3 dynamic filters).

out[b,ho,wo,f] = sum_k patches[b,ho,wo,k] * meta_w[b,ho,wo,k*3+f]
  patches[b,ho,wo,(dk0,dk1,c)] = x_pad[b, ho//2+dk0, wo//2+dk1, c]

Sharding: 8 cores, core ci handles b = ci//2, ho in [(ci%2)*64, (ci%2)*64+64).

v3 design (baseline f32/DVE-only was 193us; bf16/DVE-only 155us):
  - meta_w and the patch-row tensor xrb stream as BF16 (tolerance 2e-2, bf16
    lands ~2.4e-3), halving the dominant HBM stream 56.6 -> 28.3 MiB/core.
  - host pre-transposes mw to [WO, HO_PC*F*K] so a tile DMA is a plain 2D
    contiguous copy; tile DMAs alternate between the SP and ACT queues.
  - compute pipelines across THREE engines (DVE alone is 132us of STT):
      * DVE: fused scalar_tensor_tensor (mult+accum) for K_VF ops per 12,
        plus batched bf16 tensor_tensor multiplies (2x DVE mode) for K_MV;
      * Pool/GpSimd: batched bf16 tensor_tensor multiplies for the rest
        (walrus rejects TensorScalarPtr on Pool; tensor_tensor compiles);
      * ACT: activation(Copy, accum_out) reduces every non-fused product.
    Products flow through a PRB-deep per-tile ring; reduces of tile t gate
    producers of tile t+PRB (cmp_a), mw slot reuse gates on all three
    reader sems (cmp_v / vm_sem / pm_sem).
"""
from contextlib import ExitStack

import numpy as np

import concourse.bass as bass
import concourse.mybir as mybir
from concourse.bass_utils import run_bass_kernel_spmd

B, H, W, C = 4, 64, 64, 64
HO, WO, F = 128, 128, 3
KS = 3
K = KS * KS * C            # 576
QF = K * F                 # 1728 meta_w channels
RW = KS * C                # 192 floats per patch row (dk1, c)
N_CORES = 8
CORES_PER_B = N_CORES // B         # 2
HO_PC = HO // CORES_PER_B          # 64 output rows per core
NHS = HO_PC // 2                   # 32 hs tiles per core
NROWS = NHS + 2                    # 34 cached padded x rows per core

import os

NBUF = int(os.environ.get("K_NBUF", "4"))    # meta_w double-buffer slots
RPT = int(os.environ.get("K_RPT", "4"))      # meta_w rows per DMA tile
NSCR = int(os.environ.get("K_NSCR", "2"))    # fused-op scratch ring slots
PRB = int(os.environ.get("K_PRB", "4"))      # product ring depth (tiles)
VF12 = int(os.environ.get("K_VF", "6"))      # fused-on-DVE ops per 12
MV12 = int(os.environ.get("K_MV", "6"))      # DVE-mult ops per 12 (rest: Pool)
TAILF = int(os.environ.get("K_TAILF", "2"))  # last tiles fully fused
ALT = os.environ.get("K_ALT", "1") == "1"    # alternate SP/ACT mw queues
MFIRST = os.environ.get("K_MFIRST", "1") == "1"  # DVE mults before fused ops
# xrb head rows in the first chunk; XH=8 covers compute tiles 0-2 so the
# rest (issued on SP after tile 0) always lands before it is needed
XH = int(os.environ.get("K_XH", "8"))

f32 = mybir.dt.float32
bf16 = mybir.dt.bfloat16

if os.environ.get("K_RAMP", "1") == "1":
    # small head tiles: the first DVE op only needs a 0.88MiB transfer, so
    # compute starts ~6us earlier; small tail tiles shorten the drain.
    SCHED = [2, 2] + [RPT] * ((HO_PC - 8) // RPT) + [2, 1, 1]
elif os.environ.get("K_TAIL", "1") == "1":
    SCHED = [RPT] * ((HO_PC - 4) // RPT) + [2, 1, 1]
else:
    SCHED = [RPT] * (HO_PC // RPT)
assert sum(SCHED) == HO_PC
NT = len(SCHED)
ROW0 = [sum(SCHED[:t]) for t in range(NT)]          # first ho row of tile t
MAXOPS = max(SCHED) * F

# ---------------------------------------------------------------------------
# Static per-tile plan: op o = (r, f) at slot offset o*K; first VF ops fused
# on DVE, next MV multiplied on DVE, rest multiplied on Pool; every non-fused
# op is reduced on ACT. Mult batches group consecutive ops sharing a window.
PLAN = []      # per tile: dict(vf, mv_batches, mp_batches, n)
for t in range(NT):
    n = SCHED[t] * F
    if t >= NT - TAILF:
        vf, mv = n, 0
    else:
        vf = min(n, (n * VF12 + 6) // 12)
        mv = min(n - vf, (n * MV12 + 6) // 12)

    def _batches(lo, hi):
        """group ops [lo, hi) into runs sharing a window"""
        runs = []
        o = lo
        while o < hi:
            m = (ROW0[t] + o // F) // 2
            e = o
            while e < hi and (ROW0[t] + e // F) // 2 == m:
                e += 1
            runs.append((o, e, m))
            o = e
        return runs

    PLAN.append(
        dict(
            n=n,
            vf=vf,
            mv=mv,
            mvb=_batches(vf, vf + mv),
            mpb=_batches(vf + mv, n),
        )
    )

VF_CUM = [0]   # fused DVE ops completed through tile t-1
VMB_CUM = [0]  # DVE mult batches
PMB_CUM = [0]  # Pool mult batches
A_CUM = [0]    # ACT reduces
for t in range(NT):
    p = PLAN[t]
    VF_CUM.append(VF_CUM[-1] + p["vf"])
    VMB_CUM.append(VMB_CUM[-1] + len(p["mvb"]))
    PMB_CUM.append(PMB_CUM[-1] + len(p["mpb"]))
    A_CUM.append(A_CUM[-1] + (p["n"] - p["vf"]))

# tiles covering rows [0, 60): used for the early output-store overlap
T60 = next(t for t in range(NT + 1) if sum(SCHED[:t]) >= 60)

# product-ring slot size: widest per-tile product set (ops that aren't fused)
PS = max(p["n"] - p["vf"] for p in PLAN)

_CACHED = None


def _build_nc():
    # Cross-engine ordering is fully explicit via semaphores below; the
    # remaining same-engine WAW (scratch rings) is safe on HW because each
    # engine drains its pipe between ops, so skip the detector's extra waits.
    nc = bass.Bass(detect_race_conditions=False)
    mw_d = nc.declare_dram_parameter("mw", [WO, HO_PC * QF], bf16, isOutput=False)
    xrb_d = nc.declare_dram_parameter("xrb", [WO, NROWS * RW], bf16, isOutput=False)
    out_d = nc.declare_dram_parameter("out", [WO, HO_PC * F], f32, isOutput=True)

    with ExitStack() as ctx:
        xrow = ctx.enter_context(nc.sbuf_tensor([WO, NROWS * RW], bf16))
        mwbuf = ctx.enter_context(nc.sbuf_tensor([WO, NBUF * RPT * QF], bf16))
        prod = ctx.enter_context(nc.sbuf_tensor([WO, PRB * PS * K], bf16))
        scr_v = ctx.enter_context(nc.sbuf_tensor([WO, NSCR * K], bf16))
        ascr = ctx.enter_context(nc.sbuf_tensor([WO, NSCR * K], bf16))
        out_sb = ctx.enter_context(nc.sbuf_tensor([WO, HO_PC * F], f32))
        slot_sem = [ctx.enter_context(nc.semaphore(f"slot{j}")) for j in range(NBUF)]
        xh_sem = ctx.enter_context(nc.semaphore("xh"))      # xrb head landed
        misc_sem = ctx.enter_context(nc.semaphore("misc"))  # xrb rest landed
        cmp_v = ctx.enter_context(nc.semaphore("cmp_v"))   # DVE fused ops done
        cmp_a = ctx.enter_context(nc.semaphore("cmp_a"))   # ACT reduces done
        vm_sem = ctx.enter_context(nc.semaphore("vm"))     # DVE mult batches
        pm_sem = ctx.enter_context(nc.semaphore("pm"))     # Pool mult batches
        # Pool's dge_drain at block exit is expensive; skip it when no pool
        # ops are emitted (pm_sem waits order everything that matters).
        block = ctx.enter_context(nc.Block(no_gpsimd_drain=PMB_CUM[NT] == 0))

        def slot_ap(j, rows):
            base = j * RPT * QF
            return mwbuf[:, base : base + rows * QF]

        def win_ap(m, nb):
            w = xrow[:, m * RW : m * RW + KS * RW]
            if nb == 1:
                return w.rearrange("p (o k) -> p o k", o=1)
            return w.rearrange("p (o k) -> p o k", o=1).to_broadcast([WO, nb, K])

        def prod_op_ap(t, o):
            base = (t % PRB) * PS * K + (o - PLAN[t]["vf"]) * K
            return prod[:, base : base + K]

        def prod_run_ap(t, o0, o1):
            base = (t % PRB) * PS * K - PLAN[t]["vf"] * K
            return prod[:, base + o0 * K : base + o1 * K].rearrange(
                "p (o k) -> p o k", o=o1 - o0
            )

        def issue_tile(eng, i):
            j = i % NBUF
            rows, row0 = SCHED[i], ROW0[i]
            if i >= NBUF:
                # all three mw readers finished with the slot's previous tile
                prev = i - NBUF
                if VF_CUM[prev + 1] > 0:
                    eng.wait_ge(cmp_v, VF_CUM[prev + 1])
                if VMB_CUM[prev + 1] > 0:
                    eng.wait_ge(vm_sem, VMB_CUM[prev + 1])
                if PMB_CUM[prev + 1] > 0:
                    eng.wait_ge(pm_sem, PMB_CUM[prev + 1])
            eng.dma_start(
                out=slot_ap(j, rows),
                in_=mw_d[:, row0 * QF : (row0 + rows) * QF],
            ).then_inc(slot_sem[j], 16)

        def act_owned(i):
            return ALT and i % 2 == 1

        @block.sync
        def _(sync):
            nsp = 0
            for i in range(NT):
                if act_owned(i):
                    continue
                issue_tile(sync, i)
                nsp += 1
                if nsp == 2:
                    # xrb rest behind the first two SP tiles: lands ~16us in,
                    # well before compute reaches rows beyond the XH-row head
                    sync.dma_start(
                        out=xrow[:, XH * RW :], in_=xrb_d[:, XH * RW :]
                    ).then_inc(misc_sem, 16)
            # overlap the bulk of the output store with the tail tiles
            if VF_CUM[T60] > 0:
                sync.wait_ge(cmp_v, VF_CUM[T60])
            if A_CUM[T60] > 0:
                sync.wait_ge(cmp_a, A_CUM[T60])
            sync.dma_start(
                out=out_d[:, : 60 * F], in_=out_sb[:, : 60 * F]
            ).then_inc(misc_sem, 16)
            if VF_CUM[NT] > 0:
                sync.wait_ge(cmp_v, VF_CUM[NT])
            if A_CUM[NT] > 0:
                sync.wait_ge(cmp_a, A_CUM[NT])
            sync.dma_start(
                out=out_d[:, 60 * F :], in_=out_sb[:, 60 * F :]
            ).then_inc(misc_sem, 16)

        @block.scalar
        def _(scalar):
            # xrb head first so compute can start after ~0.4MiB; ACT-owned
            # tiles < NBUF next (no wait conditions).
            scalar.dma_start(
                out=xrow[:, : XH * RW], in_=xrb_d[:, : XH * RW]
            ).then_inc(xh_sem, 16)
            for i in range(min(NBUF, NT)):
                if act_owned(i):
                    issue_tile(scalar, i)
            # interleave reduces with this queue's remaining tile issues
            vm_seen = pm_seen = 0
            na = 0
            for t in range(NT):
                p = PLAN[t]
                # pool products land earliest; reduce them first, then DVE's
                order = [(o0, o1, "p") for (o0, o1, _m) in p["mpb"]] + [
                    (o0, o1, "v") for (o0, o1, _m) in p["mvb"]
                ]
                for o0, o1, src in order:
                    if src == "p":
                        pm_seen += 1
                        scalar.wait_ge(pm_sem, pm_seen)
                    else:
                        vm_seen += 1
                        scalar.wait_ge(vm_sem, vm_seen)
                    for o in range(o0, o1):
                        r, f = divmod(o, F)
                        ho = ROW0[t] + r
                        scalar.activation(
                            out=ascr[:, (na % NSCR) * K : (na % NSCR + 1) * K],
                            in_=prod_op_ap(t, o),
                            func=mybir.ActivationFunctionType.Copy,
                            accum_out=out_sb[:, ho * F + f : ho * F + f + 1],
                        ).then_inc(cmp_a, 1)
                        na += 1
                i2 = t + NBUF
                if i2 < NT and act_owned(i2):
                    issue_tile(scalar, i2)

        @block.vector
        def _(vector):
            vector.wait_ge(xh_sem, 16)
            nv = 0
            xrow_full_waited = False
            for t in range(NT):
                p = PLAN[t]
                j, pg = t % NBUF, t // NBUF
                rows = SCHED[t]
                if (
                    not xrow_full_waited
                    and (ROW0[t] + rows - 1) // 2 + 2 >= XH
                ):
                    vector.wait_ge(misc_sem, 16)  # rest of xrow loaded
                    xrow_full_waited = True
                vector.wait_ge(slot_sem[j], 16 * (pg + 1))
                mw3 = slot_ap(j, rows).rearrange("p (o k) -> p o k", o=rows * F)
                def do_mults():
                    if p["mvb"]:
                        if t >= PRB and A_CUM[t - PRB + 1] > 0:
                            vector.wait_ge(cmp_a, A_CUM[t - PRB + 1])
                        for o0, o1, m in p["mvb"]:
                            vector.tensor_tensor(
                                out=prod_run_ap(t, o0, o1),
                                in0=mw3[:, o0:o1, :],
                                in1=win_ap(m, o1 - o0),
                                op=mybir.AluOpType.mult,
                            ).then_inc(vm_sem, 1)

                # multiplies first: ACT can start this tile's reduces while
                # the fused ops below are still running
                if MFIRST:
                    do_mults()
                for o in range(p["vf"]):
                    r, f = divmod(o, F)
                    ho = ROW0[t] + r
                    vector.scalar_tensor_tensor(
                        out=scr_v[:, (nv % NSCR) * K : (nv % NSCR + 1) * K],
                        in0=mw3[:, o, :],
                        scalar=1.0,
                        in1=xrow[:, (ho // 2) * RW : (ho // 2) * RW + KS * RW],
                        op0=mybir.AluOpType.mult,
                        op1=mybir.AluOpType.mult,
                        accum_out=out_sb[:, ho * F + f : ho * F + f + 1],
                    ).then_inc(cmp_v, 1)
                    nv += 1
                if not MFIRST:
                    do_mults()

        if PMB_CUM[NT] > 0:

            @block.gpsimd
            def _(gpsimd):
                gpsimd.wait_ge(xh_sem, 16)
                xrow_full_waited = False
                for t in range(NT):
                    p = PLAN[t]
                    if not p["mpb"]:
                        continue
                    j, pg = t % NBUF, t // NBUF
                    rows = SCHED[t]
                    if (
                        not xrow_full_waited
                        and (ROW0[t] + rows - 1) // 2 + 2 >= XH
                    ):
                        gpsimd.wait_ge(misc_sem, 16)
                        xrow_full_waited = True
                    gpsimd.wait_ge(slot_sem[j], 16 * (pg + 1))
                    if t >= PRB and A_CUM[t - PRB + 1] > 0:
                        gpsimd.wait_ge(cmp_a, A_CUM[t - PRB + 1])
                    mw3 = slot_ap(j, rows).rearrange(
                        "p (o k) -> p o k", o=rows * F
                    )
                    for o0, o1, m in p["mpb"]:
                        gpsimd.tensor_tensor(
                            out=prod_run_ap(t, o0, o1),
                            in0=mw3[:, o0:o1, :],
                            in1=win_ap(m, o1 - o0),
                            op=mybir.AluOpType.mult,
                        ).then_inc(pm_sem, 1)

    return nc


def _prep_xrb(x):
    """Per-core duplicated patch-row tensors (bf16).

    xrb[ci][wo, hpl*RW + dk1*C + c] = x_pad[b, hs0+hpl, wo//2 + dk1, c]
    where x_pad has 1 zero row/col of padding on each side.
    """
    import ml_dtypes
    from numpy.lib.stride_tricks import sliding_window_view

    out = []
    for ci in range(N_CORES):
        b, hs0 = ci // CORES_PER_B, (ci % CORES_PER_B) * NHS
        xp = np.pad(x[b], ((1, 1), (1, 1), (0, 0)))          # [66, 66, 64]
        rows = xp[hs0 : hs0 + NROWS]                          # [34, 66, 64]
        win = sliding_window_view(rows, KS, axis=1)           # [34, 64(ws), 64(c), 3(dk1)]
        win = win.transpose(0, 1, 3, 2).reshape(NROWS, W, RW)  # [34, 64, 192]
        dup = np.repeat(win, 2, axis=1)                       # [34, 128, 192]
        out.append(
            np.ascontiguousarray(dup.transpose(1, 0, 2))
            .reshape(WO, NROWS * RW)
            .astype(ml_dtypes.bfloat16)
        )
    return out


def _ensure_axon_hooks_module():
    """This image's antenv lacks axon_hooks; run_bass_kernel_spmd imports it
    when BASS_TRACE is set. Provide it (registering the real NTFF hook when
    available) so tracing degrades gracefully instead of crashing."""
    try:
        import antenv.axon_hooks  # noqa: F401
        return
    except ImportError:
        pass
    import sys
    import types

    try:
        import antenv
    except ImportError:
        return
    mod = types.ModuleType("antenv.axon_hooks")
    _hook = [None]
    mod.set_axon_ntff_profile_hook = lambda h: _hook.__setitem__(0, h)
    mod.get_axon_ntff_profile_hook = lambda: _hook[0]
    sys.modules["antenv.axon_hooks"] = mod
    antenv.axon_hooks = mod
    try:
        from trn_agent_boot.trn_boot import _ntff_profile_via_ctypes

        h = _ntff_profile_via_ctypes("/opt/axon/libaxon_pjrt.so")
        if h is not None:
            _hook[0] = h
    except Exception:
        pass


_ensure_axon_hooks_module()

last_results = None  # BassKernelResults of the most recent kernel() call


def kernel(x, meta_w):
    global _CACHED, last_results
    import ml_dtypes

    x = np.ascontiguousarray(np.asarray(x, dtype=np.float32))
    meta_w = np.asarray(meta_w, dtype=np.float32)

    if _CACHED is None:
        _CACHED = _build_nc()
    nc = _CACHED

    xrbs = _prep_xrb(x)
    mw16 = meta_w.astype(ml_dtypes.bfloat16)
    in_maps = []
    for ci in range(N_CORES):
        b, ho0 = ci // CORES_PER_B, (ci % CORES_PER_B) * HO_PC
        # [HO_PC, WO, K, F] -> [WO, HO_PC, F, K] so a tile DMA is one plain
        # 2D contiguous copy and the per-op in0 read is contiguous (F-major).
        mw_c = np.ascontiguousarray(
            mw16[b, ho0 : ho0 + HO_PC].reshape(HO_PC, WO, K, F).transpose(1, 0, 3, 2)
        ).reshape(WO, HO_PC * QF)
        in_maps.append({"mw": mw_c, "xrb": xrbs[ci]})

    res = run_bass_kernel_spmd(nc, in_maps, list(range(N_CORES)))
    last_results = res

    out = np.empty((B, HO, WO, F), np.float32)
    for ci in range(N_CORES):
        b, ho0 = ci // CORES_PER_B, (ci % CORES_PER_B) * HO_PC
        o = res.results[ci]["out"].reshape(WO, HO_PC, F)
        out[b, ho0 : ho0 + HO_PC] = o.transpose(1, 0, 2)
    return out
